# revision 9
# baseline (speedup 1.0000x reference)
"""Expert-parallel MoE feed-forward for Trainium2 (8 NeuronCores).

Strategy (v4):
  - Host: gate + top-2 routing (0.02% of FLOPs), builds per-expert token
    index lists.  Expert e is owned by core e.
  - Device (same SPMD program on all 8 cores): indirect-DMA gather of the
    expert's tokens (bf16), DMA-XBAR transpose to [d, tok] layout, FFN
    y = relu(x@W1+b1)@W2+b2 in bf16 (full PE rate, fp32 PSUM accumulate),
    scale by combine weight, write compact [C, D] fp32 result.
  - Host: scatter-add compact results into the [B,S,D] output.

Design notes:
  - bf16 operands everywhere on the PE (same 1 cycle/row as f32r, half the
    DMA/SBUF footprint) -> single pass over all C tokens, so W1 and W2
    stream from HBM exactly once (16MB instead of 64MB).
  - x transposes on the DMA XBAR (dma_start_transpose); PE runs only matmuls.
  - W2 resident in SBUF (64KB/partition), loaded behind the gathers on the
    pool queue -> mm2 has no DMA dependence.
  - PE stationary operand changes on every matmul (consecutive reuse of the
    same stationary measures ~2x per-instruction overhead on TRN2).
  - mm1 PSUM drains on the vector engine: one fused (acc+b1) max 0 per chunk.
  - C = 1088 (max expert load this routing is 1075): third mm1 chunk is 64
    wide, last mm2 token tile is 64 tall.
  - Disjoint PSUM tags: mm1 accumulators (4 banks) + mm2 accumulators (4).

Layouts (per expert e / core e):
  xT[p, k, t]    = x[tok(t), k*128+p]          (DMA-XBAR-transposed gather)
  h [p, j, t]    = relu(x @ W1 + b1)[tok(t), j*128+p]
  w1_d[p, j, k, c] = W1[e][k*128+p, j*128+c]   (host-swizzled, bf16)
  w2_d[p, j, d]  = W2[e][j*128+p, d]           (host-swizzled, bf16)
  mm1: h[:, j, chunk] = sum_k w1t[:, k, :].T @ xT[:, k, chunk]
  mm2: y[m-tile, dhalf] = sum_j h[:, j, m-tile].T @ w2sb[:, j, dhalf]
"""

import numpy as np

B, S, D, F, E = 2, 2048, 1024, 4096, 8
T = B * S                      # 4096 tokens
K_TOP = 2
C = 1088                       # per-expert token capacity (8.5 * 128)
P = 128
KD = D // P                    # 8  k-tiles (mm1 contraction)
NJ = F // P                    # 32 f-tiles
NG = 9                         # gather tiles of 128 (covers C with padding)
DH = D // 2                    # 512 (mm2 moving width)
CHUNKS = [(0, 512), (512, 512), (1024, 64)]    # mm1 token chunks
MTILES = [(m * P, P) for m in range(8)] + [(1024, 64)]  # mm2 token tiles

_CACHE = {}


def _build_program(reps=1, loop_n=1):
    import concourse.bass as bass
    import concourse.mybir as mybir
    import concourse.tile as tile
    from concourse import bacc
    from contextlib import ExitStack

    f32 = mybir.dt.float32
    bf16 = mybir.dt.bfloat16
    i32 = mybir.dt.int32

    nc = bacc.Bacc("TRN2", target_bir_lowering=False, debug=False)

    x_d = nc.dram_tensor("x", [T, D], bf16, kind="ExternalInput").ap()
    w1_d = nc.dram_tensor("W1s", [P, NJ, KD, P], bf16, kind="ExternalInput").ap()
    w2_d = nc.dram_tensor("W2s", [P, NJ, D], bf16, kind="ExternalInput").ap()
    idx_d = nc.dram_tensor("idx", [P, NG], i32, kind="ExternalInput").ap()
    wc_d = nc.dram_tensor("wc", [P, NG], f32, kind="ExternalInput").ap()
    b1_d = nc.dram_tensor("b1t", [P, NJ], f32, kind="ExternalInput").ap()
    # b2 replicated across partitions for the free-axis bias add
    b2_d = nc.dram_tensor("b2r", [P, D], f32, kind="ExternalInput").ap()
    y_d = nc.dram_tensor("yout", [C, D], f32, kind="ExternalOutput").ap()

    with tile.TileContext(nc) as tc, ExitStack() as ctx:
        sb = ctx.enter_context(tc.tile_pool(name="sb", bufs=1))
        ps = ctx.enter_context(tc.tile_pool(name="ps", bufs=1, space="PSUM"))

        idx_t = sb.tile([P, NG], i32, tag="idx")
        wc_t = sb.tile([P, NG], f32, tag="wc")
        b1_t = sb.tile([P, NJ], f32, tag="b1")
        b2_t = sb.tile([P, D], f32, tag="b2")
        nc.sync.dma_start(idx_t[:], idx_d[:])
        nc.sync.dma_start(wc_t[:], wc_d[:])
        nc.sync.dma_start(b1_t[:], b1_d[:])
        nc.sync.dma_start(b2_t[:], b2_d[:])

        # W2 resident in SBUF for the whole kernel (64KB/partition)
        w2sb = sb.tile([P, NJ, D], bf16, tag="w2r")

        loop_cm = tc.For_i(0, loop_n, 1) if loop_n > 1 else None
        if loop_cm is not None:
            loop_cm.__enter__()

        for rep in range(reps):
            # --- gather + DMA-XBAR transpose ---
            xT = sb.tile([P, KD, NG * P], bf16, tag="xT", bufs=1,
                         name=f"xT_{rep}")
            for g in range(NG):
                xg = sb.tile([P, D], bf16, tag="xg", bufs=5,
                             name=f"xg_{rep}_{g}")
                nc.gpsimd.indirect_dma_start(
                    out=xg[:], out_offset=None,
                    in_=x_d[:],
                    in_offset=bass.IndirectOffsetOnAxis(
                        ap=idx_t[:, g:g + 1], axis=0),
                )
                for k in range(KD):
                    nc.scalar.dma_start_transpose(
                        xT[:, k, g * P:(g + 1) * P],
                        xg[:, k * P:(k + 1) * P])

            if rep == 0:
                # behind the gathers on the pool queue; needed only by mm2
                for q in range(4):
                    nc.gpsimd.dma_start(w2sb[:, q * 8:(q + 1) * 8, :],
                                        w2_d[:, q * 8:(q + 1) * 8, :])

            # --- mm1 + relu:  h[:, j, t] = relu(sum_k w1.T @ xT + b1) ---
            h = sb.tile([P, NJ, C], bf16, tag="h", bufs=1, name=f"h_{rep}")
            for j in range(NJ):
                w1t = sb.tile([P, KD, P], bf16, tag="w1", bufs=12,
                              name=f"w1_{rep}_{j}")
                nc.sync.dma_start(w1t[:], w1_d[:, j])
                accs = []
                for ci, (t0, tn) in enumerate(CHUNKS):
                    accs.append(ps.tile([P, tn], f32, tag="p1", bufs=4,
                                        name=f"p1_{rep}_{j}_{ci}",
                                        padded_shape=[P, 512]))
                # k inner so the PE stationary changes on every matmul
                for ci, (t0, tn) in enumerate(CHUNKS):
                    for k in range(KD):
                        nc.tensor.matmul(
                            accs[ci][:],
                            lhsT=w1t[:, k, :],
                            rhs=xT[:, k, t0:t0 + tn],
                            start=(k == 0), stop=(k == KD - 1))
                for ci, (t0, tn) in enumerate(CHUNKS):
                    # fused (acc + b1) max 0 -> bf16, on the vector engine
                    nc.vector.tensor_scalar(
                        out=h[:, j, t0:t0 + tn], in0=accs[ci][:],
                        scalar1=b1_t[:, j:j + 1], scalar2=0.0,
                        op0=mybir.AluOpType.add, op1=mybir.AluOpType.max)

            # --- mm2: y[m-tile, :] = sum_j h[:, j, m-tile].T @ w2sb[:, j, :] ---
            for mi, (m0, mn) in enumerate(MTILES):
                acc2 = [ps.tile([mn, DH], f32, tag="p2", bufs=4,
                                name=f"p2_{rep}_{mi}_{dn}",
                                padded_shape=[P, DH]) for dn in range(2)]
                # j inner: stationary (h block) changes on every matmul
                for dn in range(2):
                    for j in range(NJ):
                        nc.tensor.matmul(
                            acc2[dn][:],
                            lhsT=h[:, j, m0:m0 + mn],
                            rhs=w2sb[:, j, dn * DH:(dn + 1) * DH],
                            start=(j == 0), stop=(j == NJ - 1))
                for dn in range(2):
                    ot = sb.tile([mn, DH], f32, tag="ot", bufs=4,
                                 name=f"ot_{rep}_{mi}_{dn}",
                                 padded_shape=[P, DH])
                    nc.vector.tensor_tensor(
                        out=ot[:], in0=acc2[dn][:],
                        in1=b2_t[:mn, dn * DH:(dn + 1) * DH],
                        op=mybir.AluOpType.add)
                    nc.vector.tensor_scalar_mul(
                        ot[:], ot[:], wc_t[:mn, mi:mi + 1])
                    nc.scalar.dma_start(
                        y_d[m0:m0 + mn, dn * DH:(dn + 1) * DH], ot[:])

        if loop_cm is not None:
            loop_cm.__exit__(None, None, None)

    nc.compile()
    return nc


def _route(x2, Wg, bg):
    """Host-side top-2 routing in float64 (stable ordering)."""
    gate = x2.astype(np.float64) @ np.asarray(Wg, np.float64) + np.asarray(bg, np.float64)
    part = np.argpartition(-gate, K_TOP - 1, axis=1)[:, :K_TOP]      # [T, 2]
    rows = np.arange(T)[:, None]
    sc = gate[rows, part]                                            # [T, 2]
    sc = sc - sc.max(axis=1, keepdims=True)
    e_sc = np.exp(sc)
    probs = e_sc / e_sc.sum(axis=1, keepdims=True)                   # [T, 2]
    idx_e, w_e, n_e = [], [], []
    for e in range(E):
        mask = part == e                                             # [T, 2]
        tok = np.nonzero(mask.any(axis=1))[0]
        pr = probs[mask]                                             # aligned with tok
        n = len(tok)
        pad = NG * P - n
        if n > C:
            return None                                              # capacity overflow
        idx_e.append(np.concatenate([tok, np.zeros(pad, np.int64)]).astype(np.int32))
        w_e.append(np.concatenate([pr, np.zeros(pad)]).astype(np.float32))
        n_e.append(n)
    return idx_e, w_e, n_e


def _mk_in_maps(x2, W1, b1, W2, b2, idx_e, w_e):
    """Build per-core device input dicts (bf16 swizzled weights)."""
    import ml_dtypes
    bf16 = ml_dtypes.bfloat16

    x_b = np.ascontiguousarray(x2.astype(bf16))
    in_maps = []
    for e in range(E):
        # W1[e] [D, F] -> [P(p), NJ(j), KD(k), P(c)]
        w1s = np.ascontiguousarray(
            W1[e].reshape(KD, P, NJ, P).transpose(1, 2, 0, 3).astype(bf16))
        # W2[e] [F, D] -> [P(p), NJ(j), D]
        w2s = np.ascontiguousarray(
            W2[e].reshape(NJ, P, D).transpose(1, 0, 2).astype(bf16))
        in_maps.append({
            "x": x_b,
            "W1s": w1s,
            "W2s": w2s,
            "idx": np.ascontiguousarray(idx_e[e].reshape(NG, P).T),
            "wc": np.ascontiguousarray(w_e[e].reshape(NG, P).T),
            "b1t": np.ascontiguousarray(b1[e].reshape(NJ, P).T),
            "b2r": np.ascontiguousarray(np.broadcast_to(b2[e], (P, D))),
        })
    return in_maps


def kernel(x, W1, b1, W2, b2, Wg, bg, num_experts_per_token):
    from concourse.bass_utils import run_bass_kernel_spmd

    x2 = np.ascontiguousarray(np.asarray(x, np.float32).reshape(T, D))
    W1 = np.asarray(W1, np.float32)
    b1 = np.asarray(b1, np.float32)
    W2 = np.asarray(W2, np.float32)
    b2 = np.asarray(b2, np.float32)

    routing = _route(x2, Wg, bg)
    if routing is None or int(num_experts_per_token) != K_TOP:
        # capacity overflow or unexpected top-k: correct slow path
        gate = x2.astype(np.float64) @ np.asarray(Wg, np.float64) + np.asarray(bg, np.float64)
        k = int(num_experts_per_token)
        part = np.argsort(-gate, axis=1)[:, :k]
        sc = gate[np.arange(T)[:, None], part]
        sc = sc - sc.max(axis=1, keepdims=True)
        pr = np.exp(sc); pr /= pr.sum(axis=1, keepdims=True)
        out = np.zeros((T, D), np.float32)
        for e in range(E):
            mask = part == e
            tok = np.nonzero(mask.any(axis=1))[0]
            w = pr[mask].astype(np.float32)
            hcur = np.maximum(x2[tok] @ W1[e] + b1[e], 0.0)
            out[tok] += w[:, None] * (hcur @ W2[e] + b2[e])
        return out.reshape(B, S, D)

    idx_e, w_e, n_e = routing

    if "nc" not in _CACHE:
        _CACHE["nc"] = _build_program()
    nc = _CACHE["nc"]

    in_maps = _mk_in_maps(x2, W1, b1, W2, b2, idx_e, w_e)
    res = run_bass_kernel_spmd(nc, in_maps, list(range(E)))

    out = np.zeros((T, D), np.float32)
    for e in range(E):
        n = n_e[e]
        out[idx_e[e][:n]] += res.results[e]["yout"][:n]
    return out.reshape(B, S, D)


# revision 10
# speedup vs baseline: 1.0230x; 1.0230x over previous
"""Expert-parallel MoE feed-forward for Trainium2 (8 NeuronCores).

Strategy (v4):
  - Host: gate + top-2 routing (0.02% of FLOPs), builds per-expert token
    index lists.  Expert e is owned by core e.
  - Device (same SPMD program on all 8 cores): indirect-DMA gather of the
    expert's tokens (bf16), DMA-XBAR transpose to [d, tok] layout, FFN
    y = relu(x@W1+b1)@W2+b2 in bf16 (full PE rate, fp32 PSUM accumulate),
    scale by combine weight, write compact [C, D] fp32 result.
  - Host: scatter-add compact results into the [B,S,D] output.

Design notes:
  - bf16 operands everywhere on the PE (same 1 cycle/row as f32r, half the
    DMA/SBUF footprint) -> single pass over all C tokens, so W1 and W2
    stream from HBM exactly once (16MB instead of 64MB).
  - x transposes on the DMA XBAR (dma_start_transpose); PE runs only matmuls.
  - W2 resident in SBUF (64KB/partition), loaded behind the gathers on the
    pool queue -> mm2 has no DMA dependence.
  - PE stationary operand changes on every matmul (consecutive reuse of the
    same stationary measures ~2x per-instruction overhead on TRN2).
  - mm1 PSUM drains on the vector engine: one fused (acc+b1) max 0 per chunk.
  - C = 1088 (max expert load this routing is 1075): third mm1 chunk is 64
    wide, last mm2 token tile is 64 tall.
  - Disjoint PSUM tags: mm1 accumulators (4 banks) + mm2 accumulators (4).

Layouts (per expert e / core e):
  xT[p, k, t]    = x[tok(t), k*128+p]          (DMA-XBAR-transposed gather)
  h [p, j, t]    = relu(x @ W1 + b1)[tok(t), j*128+p]
  w1_d[p, j, k, c] = W1[e][k*128+p, j*128+c]   (host-swizzled, bf16)
  w2_d[p, j, d]  = W2[e][j*128+p, d]           (host-swizzled, bf16)
  mm1: h[:, j, chunk] = sum_k w1t[:, k, :].T @ xT[:, k, chunk]
  mm2: y[m-tile, dhalf] = sum_j h[:, j, m-tile].T @ w2sb[:, j, dhalf]
"""

import numpy as np

B, S, D, F, E = 2, 2048, 1024, 4096, 8
T = B * S                      # 4096 tokens
K_TOP = 2
C = 1088                       # per-expert token capacity (8.5 * 128)
P = 128
KD = D // P                    # 8  k-tiles (mm1 contraction)
NJ = F // P                    # 32 f-tiles
NG = 9                         # gather tiles of 128 (covers C with padding)
DH = D // 2                    # 512 (mm2 moving width)
CHUNKS = [(0, 512), (512, 512), (1024, 64)]    # mm1 token chunks
MTILES = [(m * P, P) for m in range(8)] + [(1024, 64)]  # mm2 token tiles

_CACHE = {}


def _build_program(reps=1, loop_n=1):
    import concourse.bass as bass
    import concourse.mybir as mybir
    import concourse.tile as tile
    from concourse import bacc
    from contextlib import ExitStack

    f32 = mybir.dt.float32
    bf16 = mybir.dt.bfloat16
    i32 = mybir.dt.int32

    nc = bacc.Bacc("TRN2", target_bir_lowering=False, debug=False)

    x_d = nc.dram_tensor("x", [T, D], bf16, kind="ExternalInput").ap()
    w1_d = nc.dram_tensor("W1s", [P, NJ, KD, P], bf16, kind="ExternalInput").ap()
    w2_d = nc.dram_tensor("W2s", [P, NJ, D], bf16, kind="ExternalInput").ap()
    idx_d = nc.dram_tensor("idx", [P, NG], i32, kind="ExternalInput").ap()
    wc_d = nc.dram_tensor("wc", [P, NG], f32, kind="ExternalInput").ap()
    b1_d = nc.dram_tensor("b1t", [P, NJ], f32, kind="ExternalInput").ap()
    # b2 replicated across partitions for the free-axis bias add
    b2_d = nc.dram_tensor("b2r", [P, D], f32, kind="ExternalInput").ap()
    y_d = nc.dram_tensor("yout", [C, D], f32, kind="ExternalOutput").ap()

    with tile.TileContext(nc) as tc, ExitStack() as ctx:
        sb = ctx.enter_context(tc.tile_pool(name="sb", bufs=1))
        ps = ctx.enter_context(tc.tile_pool(name="ps", bufs=1, space="PSUM"))

        idx_t = sb.tile([P, NG], i32, tag="idx")
        wc_t = sb.tile([P, NG], f32, tag="wc")
        b1_t = sb.tile([P, NJ], f32, tag="b1")
        b2_t = sb.tile([P, D], f32, tag="b2")
        nc.sync.dma_start(idx_t[:], idx_d[:])
        nc.sync.dma_start(wc_t[:], wc_d[:])
        nc.sync.dma_start(b1_t[:], b1_d[:])
        nc.sync.dma_start(b2_t[:], b2_d[:])

        # W2 resident in SBUF for the whole kernel (64KB/partition)
        w2sb = sb.tile([P, NJ, D], bf16, tag="w2r")

        loop_cm = tc.For_i(0, loop_n, 1) if loop_n > 1 else None
        if loop_cm is not None:
            loop_cm.__enter__()

        for rep in range(reps):
            # --- gather + DMA-XBAR transpose ---
            xT = sb.tile([P, KD, NG * P], bf16, tag="xT", bufs=1,
                         name=f"xT_{rep}")
            for g in range(NG):
                xg = sb.tile([P, D], bf16, tag="xg", bufs=5,
                             name=f"xg_{rep}_{g}")
                nc.gpsimd.indirect_dma_start(
                    out=xg[:], out_offset=None,
                    in_=x_d[:],
                    in_offset=bass.IndirectOffsetOnAxis(
                        ap=idx_t[:, g:g + 1], axis=0),
                )
                for k in range(KD):
                    nc.scalar.dma_start_transpose(
                        xT[:, k, g * P:(g + 1) * P],
                        xg[:, k * P:(k + 1) * P])

            if rep == 0:
                # behind the gathers on the pool queue; needed only by mm2
                for q in range(4):
                    nc.gpsimd.dma_start(w2sb[:, q * 8:(q + 1) * 8, :],
                                        w2_d[:, q * 8:(q + 1) * 8, :])

            # --- mm1 + relu:  h[:, j, t] = relu(sum_k w1.T @ xT + b1) ---
            h = sb.tile([P, NJ, C], bf16, tag="h", bufs=1, name=f"h_{rep}")
            for j in range(NJ):
                w1t = sb.tile([P, KD, P], bf16, tag="w1", bufs=12,
                              name=f"w1_{rep}_{j}")
                nc.sync.dma_start(w1t[:], w1_d[:, j])
                accs = []
                for ci, (t0, tn) in enumerate(CHUNKS):
                    accs.append(ps.tile([P, tn], f32, tag="p1", bufs=6,
                                        name=f"p1_{rep}_{j}_{ci}",
                                        padded_shape=[P, 512]))
                # k inner so the PE stationary changes on every matmul
                for ci, (t0, tn) in enumerate(CHUNKS):
                    for k in range(KD):
                        nc.tensor.matmul(
                            accs[ci][:],
                            lhsT=w1t[:, k, :],
                            rhs=xT[:, k, t0:t0 + tn],
                            start=(k == 0), stop=(k == KD - 1))
                for ci, (t0, tn) in enumerate(CHUNKS):
                    # fused (acc + b1) max 0 -> bf16, on the vector engine
                    nc.vector.tensor_scalar(
                        out=h[:, j, t0:t0 + tn], in0=accs[ci][:],
                        scalar1=b1_t[:, j:j + 1], scalar2=0.0,
                        op0=mybir.AluOpType.add, op1=mybir.AluOpType.max)

            # --- mm2: y[m-tile, :] = sum_j h[:, j, m-tile].T @ w2sb[:, j, :] ---
            for mi, (m0, mn) in enumerate(MTILES):
                acc2 = [ps.tile([mn, DH], f32, tag="p2", bufs=2,
                                name=f"p2_{rep}_{mi}_{dn}",
                                padded_shape=[P, DH]) for dn in range(2)]
                # j inner: stationary (h block) changes on every matmul
                for dn in range(2):
                    for j in range(NJ):
                        nc.tensor.matmul(
                            acc2[dn][:],
                            lhsT=h[:, j, m0:m0 + mn],
                            rhs=w2sb[:, j, dn * DH:(dn + 1) * DH],
                            start=(j == 0), stop=(j == NJ - 1))
                for dn in range(2):
                    ot = sb.tile([mn, DH], f32, tag="ot", bufs=4,
                                 name=f"ot_{rep}_{mi}_{dn}",
                                 padded_shape=[P, DH])
                    nc.vector.tensor_tensor(
                        out=ot[:], in0=acc2[dn][:],
                        in1=b2_t[:mn, dn * DH:(dn + 1) * DH],
                        op=mybir.AluOpType.add)
                    nc.vector.tensor_scalar_mul(
                        ot[:], ot[:], wc_t[:mn, mi:mi + 1])
                    nc.scalar.dma_start(
                        y_d[m0:m0 + mn, dn * DH:(dn + 1) * DH], ot[:])

        if loop_cm is not None:
            loop_cm.__exit__(None, None, None)

    nc.compile()
    return nc


def _route(x2, Wg, bg):
    """Host-side top-2 routing in float64 (stable ordering)."""
    gate = x2.astype(np.float64) @ np.asarray(Wg, np.float64) + np.asarray(bg, np.float64)
    part = np.argpartition(-gate, K_TOP - 1, axis=1)[:, :K_TOP]      # [T, 2]
    rows = np.arange(T)[:, None]
    sc = gate[rows, part]                                            # [T, 2]
    sc = sc - sc.max(axis=1, keepdims=True)
    e_sc = np.exp(sc)
    probs = e_sc / e_sc.sum(axis=1, keepdims=True)                   # [T, 2]
    idx_e, w_e, n_e = [], [], []
    for e in range(E):
        mask = part == e                                             # [T, 2]
        tok = np.nonzero(mask.any(axis=1))[0]
        pr = probs[mask]                                             # aligned with tok
        n = len(tok)
        pad = NG * P - n
        if n > C:
            return None                                              # capacity overflow
        idx_e.append(np.concatenate([tok, np.zeros(pad, np.int64)]).astype(np.int32))
        w_e.append(np.concatenate([pr, np.zeros(pad)]).astype(np.float32))
        n_e.append(n)
    return idx_e, w_e, n_e


def _mk_in_maps(x2, W1, b1, W2, b2, idx_e, w_e):
    """Build per-core device input dicts (bf16 swizzled weights)."""
    import ml_dtypes
    bf16 = ml_dtypes.bfloat16

    x_b = np.ascontiguousarray(x2.astype(bf16))
    in_maps = []
    for e in range(E):
        # W1[e] [D, F] -> [P(p), NJ(j), KD(k), P(c)]
        w1s = np.ascontiguousarray(
            W1[e].reshape(KD, P, NJ, P).transpose(1, 2, 0, 3).astype(bf16))
        # W2[e] [F, D] -> [P(p), NJ(j), D]
        w2s = np.ascontiguousarray(
            W2[e].reshape(NJ, P, D).transpose(1, 0, 2).astype(bf16))
        in_maps.append({
            "x": x_b,
            "W1s": w1s,
            "W2s": w2s,
            "idx": np.ascontiguousarray(idx_e[e].reshape(NG, P).T),
            "wc": np.ascontiguousarray(w_e[e].reshape(NG, P).T),
            "b1t": np.ascontiguousarray(b1[e].reshape(NJ, P).T),
            "b2r": np.ascontiguousarray(np.broadcast_to(b2[e], (P, D))),
        })
    return in_maps


def kernel(x, W1, b1, W2, b2, Wg, bg, num_experts_per_token):
    from concourse.bass_utils import run_bass_kernel_spmd

    x2 = np.ascontiguousarray(np.asarray(x, np.float32).reshape(T, D))
    W1 = np.asarray(W1, np.float32)
    b1 = np.asarray(b1, np.float32)
    W2 = np.asarray(W2, np.float32)
    b2 = np.asarray(b2, np.float32)

    routing = _route(x2, Wg, bg)
    if routing is None or int(num_experts_per_token) != K_TOP:
        # capacity overflow or unexpected top-k: correct slow path
        gate = x2.astype(np.float64) @ np.asarray(Wg, np.float64) + np.asarray(bg, np.float64)
        k = int(num_experts_per_token)
        part = np.argsort(-gate, axis=1)[:, :k]
        sc = gate[np.arange(T)[:, None], part]
        sc = sc - sc.max(axis=1, keepdims=True)
        pr = np.exp(sc); pr /= pr.sum(axis=1, keepdims=True)
        out = np.zeros((T, D), np.float32)
        for e in range(E):
            mask = part == e
            tok = np.nonzero(mask.any(axis=1))[0]
            w = pr[mask].astype(np.float32)
            hcur = np.maximum(x2[tok] @ W1[e] + b1[e], 0.0)
            out[tok] += w[:, None] * (hcur @ W2[e] + b2[e])
        return out.reshape(B, S, D)

    idx_e, w_e, n_e = routing

    if "nc" not in _CACHE:
        _CACHE["nc"] = _build_program()
    nc = _CACHE["nc"]

    in_maps = _mk_in_maps(x2, W1, b1, W2, b2, idx_e, w_e)
    res = run_bass_kernel_spmd(nc, in_maps, list(range(E)))

    out = np.zeros((T, D), np.float32)
    for e in range(E):
        n = n_e[e]
        out[idx_e[e][:n]] += res.results[e]["yout"][:n]
    return out.reshape(B, S, D)


# revision 11
# speedup vs baseline: 1.0478x; 1.0242x over previous
"""Expert-parallel MoE feed-forward for Trainium2 (8 NeuronCores).

Strategy (v4):
  - Host: gate + top-2 routing (0.02% of FLOPs), builds per-expert token
    index lists.  Expert e is owned by core e.
  - Device (same SPMD program on all 8 cores): indirect-DMA gather of the
    expert's tokens (bf16), DMA-XBAR transpose to [d, tok] layout, FFN
    y = relu(x@W1+b1)@W2+b2 in bf16 (full PE rate, fp32 PSUM accumulate),
    scale by combine weight, write compact [C, D] fp32 result.
  - Host: scatter-add compact results into the [B,S,D] output.

Design notes:
  - bf16 operands everywhere on the PE (same 1 cycle/row as f32r, half the
    DMA/SBUF footprint) -> single pass over all C tokens, so W1 and W2
    stream from HBM exactly once (16MB instead of 64MB).
  - x transposes on the DMA XBAR (dma_start_transpose); PE runs only matmuls.
  - W2 resident in SBUF (64KB/partition), loaded behind the gathers on the
    pool queue -> mm2 has no DMA dependence.
  - PE stationary operand changes on every matmul (consecutive reuse of the
    same stationary measures ~2x per-instruction overhead on TRN2).
  - mm1 PSUM drains on the vector engine: one fused (acc+b1) max 0 per chunk.
  - C = 1088 (max expert load this routing is 1075): third mm1 chunk is 64
    wide, last mm2 token tile is 64 tall.
  - Disjoint PSUM tags: mm1 accumulators (4 banks) + mm2 accumulators (4).

Layouts (per expert e / core e):
  xT[p, k, t]    = x[tok(t), k*128+p]          (DMA-XBAR-transposed gather)
  h [p, j, t]    = relu(x @ W1 + b1)[tok(t), j*128+p]
  w1_d[p, j, k, c] = W1[e][k*128+p, j*128+c]   (host-swizzled, bf16)
  w2_d[p, j, d]  = W2[e][j*128+p, d]           (host-swizzled, bf16)
  mm1: h[:, j, chunk] = sum_k w1t[:, k, :].T @ xT[:, k, chunk]
  mm2: y[m-tile, dhalf] = sum_j h[:, j, m-tile].T @ w2sb[:, j, dhalf]
"""

import numpy as np

B, S, D, F, E = 2, 2048, 1024, 4096, 8
T = B * S                      # 4096 tokens
K_TOP = 2
C = 1088                       # per-expert token capacity (8.5 * 128)
P = 128
KD = D // P                    # 8  k-tiles (mm1 contraction)
NJ = F // P                    # 32 f-tiles
NG = 9                         # gather tiles of 128 (covers C with padding)
DH = D // 2                    # 512 (mm2 moving width)
CHUNKS = [(0, 512), (512, 512), (1024, 64)]    # mm1 token chunks
MTILES = [(m * P, P) for m in range(8)] + [(1024, 64)]  # mm2 token tiles

_CACHE = {}


def _build_program(reps=1, loop_n=1):
    import concourse.bass as bass
    import concourse.mybir as mybir
    import concourse.tile as tile
    from concourse import bacc
    from contextlib import ExitStack

    f32 = mybir.dt.float32
    bf16 = mybir.dt.bfloat16
    i32 = mybir.dt.int32

    nc = bacc.Bacc("TRN2", target_bir_lowering=False, debug=False)

    x_d = nc.dram_tensor("x", [T, D], bf16, kind="ExternalInput").ap()
    w1_d = nc.dram_tensor("W1s", [P, NJ, KD, P], bf16, kind="ExternalInput").ap()
    w2_d = nc.dram_tensor("W2s", [P, NJ, D], bf16, kind="ExternalInput").ap()
    idx_d = nc.dram_tensor("idx", [P, NG], i32, kind="ExternalInput").ap()
    wc_d = nc.dram_tensor("wc", [P, NG], f32, kind="ExternalInput").ap()
    b1_d = nc.dram_tensor("b1t", [P, NJ], f32, kind="ExternalInput").ap()
    # b2 replicated across partitions for the free-axis bias add
    b2_d = nc.dram_tensor("b2r", [P, D], f32, kind="ExternalInput").ap()
    y_d = nc.dram_tensor("yout", [C, D], f32, kind="ExternalOutput").ap()

    with tile.TileContext(nc) as tc, ExitStack() as ctx:
        sb = ctx.enter_context(tc.tile_pool(name="sb", bufs=1))
        ps = ctx.enter_context(tc.tile_pool(name="ps", bufs=1, space="PSUM"))

        idx_t = sb.tile([P, NG], i32, tag="idx")
        wc_t = sb.tile([P, NG], f32, tag="wc")
        b1_t = sb.tile([P, NJ], f32, tag="b1")
        b2_t = sb.tile([P, D], f32, tag="b2")
        nc.sync.dma_start(idx_t[:], idx_d[:])
        nc.sync.dma_start(wc_t[:], wc_d[:])
        nc.sync.dma_start(b1_t[:], b1_d[:])
        nc.sync.dma_start(b2_t[:], b2_d[:])

        # W2 resident in SBUF for the whole kernel (64KB/partition)
        w2sb = sb.tile([P, NJ, D], bf16, tag="w2r")

        loop_cm = tc.For_i(0, loop_n, 1) if loop_n > 1 else None
        if loop_cm is not None:
            loop_cm.__enter__()

        for rep in range(reps):
            # --- gather + DMA-XBAR transpose ---
            xT = sb.tile([P, KD, NG * P], bf16, tag="xT", bufs=1,
                         name=f"xT_{rep}")
            for g in range(NG):
                xg = sb.tile([P, D], bf16, tag="xg", bufs=5,
                             name=f"xg_{rep}_{g}")
                nc.gpsimd.indirect_dma_start(
                    out=xg[:], out_offset=None,
                    in_=x_d[:],
                    in_offset=bass.IndirectOffsetOnAxis(
                        ap=idx_t[:, g:g + 1], axis=0),
                )
                for k in range(KD):
                    nc.scalar.dma_start_transpose(
                        xT[:, k, g * P:(g + 1) * P],
                        xg[:, k * P:(k + 1) * P])

            if rep == 0:
                # behind the gathers on the pool queue; needed only by mm2
                for q in range(4):
                    nc.gpsimd.dma_start(w2sb[:, q * 8:(q + 1) * 8, :],
                                        w2_d[:, q * 8:(q + 1) * 8, :])

            # --- mm1 + relu:  h[:, j, t] = relu(sum_k w1.T @ xT + b1) ---
            h = sb.tile([P, NJ, C], bf16, tag="h", bufs=1, name=f"h_{rep}")
            for j in range(NJ):
                w1t = sb.tile([P, KD, P], bf16, tag="w1", bufs=12,
                              name=f"w1_{rep}_{j}")
                nc.sync.dma_start(w1t[:], w1_d[:, j])
                accs = []
                for ci, (t0, tn) in enumerate(CHUNKS):
                    accs.append(ps.tile([P, tn], f32, tag="p1", bufs=4,
                                        name=f"p1_{rep}_{j}_{ci}",
                                        padded_shape=[P, 512]))
                # k inner so the PE stationary changes on every matmul
                for ci, (t0, tn) in enumerate(CHUNKS):
                    for k in range(KD):
                        nc.tensor.matmul(
                            accs[ci][:],
                            lhsT=w1t[:, k, :],
                            rhs=xT[:, k, t0:t0 + tn],
                            start=(k == 0), stop=(k == KD - 1))
                for ci, (t0, tn) in enumerate(CHUNKS):
                    # fused (acc + b1) max 0 -> bf16, on the vector engine
                    nc.vector.tensor_scalar(
                        out=h[:, j, t0:t0 + tn], in0=accs[ci][:],
                        scalar1=b1_t[:, j:j + 1], scalar2=0.0,
                        op0=mybir.AluOpType.add, op1=mybir.AluOpType.max)

            # --- mm2: y[m-tile, :] = sum_j h[:, j, m-tile].T @ w2sb[:, j, :] ---
            for mi, (m0, mn) in enumerate(MTILES):
                acc2 = [ps.tile([mn, DH], f32, tag="p2", bufs=4,
                                name=f"p2_{rep}_{mi}_{dn}",
                                padded_shape=[P, DH]) for dn in range(2)]
                # j inner: stationary (h block) changes on every matmul
                for dn in range(2):
                    for j in range(NJ):
                        nc.tensor.matmul(
                            acc2[dn][:],
                            lhsT=h[:, j, m0:m0 + mn],
                            rhs=w2sb[:, j, dn * DH:(dn + 1) * DH],
                            start=(j == 0), stop=(j == NJ - 1))
                for dn in range(2):
                    ot = sb.tile([mn, DH], f32, tag="ot", bufs=4,
                                 name=f"ot_{rep}_{mi}_{dn}",
                                 padded_shape=[P, DH])
                    nc.vector.tensor_tensor(
                        out=ot[:], in0=acc2[dn][:],
                        in1=b2_t[:mn, dn * DH:(dn + 1) * DH],
                        op=mybir.AluOpType.add)
                    nc.vector.tensor_scalar_mul(
                        ot[:], ot[:], wc_t[:mn, mi:mi + 1])
                    nc.scalar.dma_start(
                        y_d[m0:m0 + mn, dn * DH:(dn + 1) * DH], ot[:])

        if loop_cm is not None:
            loop_cm.__exit__(None, None, None)

    nc.compile()
    return nc


def _route(x2, Wg, bg):
    """Host-side top-2 routing in float64 (stable ordering)."""
    gate = x2.astype(np.float64) @ np.asarray(Wg, np.float64) + np.asarray(bg, np.float64)
    part = np.argpartition(-gate, K_TOP - 1, axis=1)[:, :K_TOP]      # [T, 2]
    rows = np.arange(T)[:, None]
    sc = gate[rows, part]                                            # [T, 2]
    sc = sc - sc.max(axis=1, keepdims=True)
    e_sc = np.exp(sc)
    probs = e_sc / e_sc.sum(axis=1, keepdims=True)                   # [T, 2]
    idx_e, w_e, n_e = [], [], []
    for e in range(E):
        mask = part == e                                             # [T, 2]
        tok = np.nonzero(mask.any(axis=1))[0]
        pr = probs[mask]                                             # aligned with tok
        n = len(tok)
        pad = NG * P - n
        if n > C:
            return None                                              # capacity overflow
        idx_e.append(np.concatenate([tok, np.zeros(pad, np.int64)]).astype(np.int32))
        w_e.append(np.concatenate([pr, np.zeros(pad)]).astype(np.float32))
        n_e.append(n)
    return idx_e, w_e, n_e


def _mk_in_maps(x2, W1, b1, W2, b2, idx_e, w_e):
    """Build per-core device input dicts (bf16 swizzled weights)."""
    import ml_dtypes
    bf16 = ml_dtypes.bfloat16

    x_b = np.ascontiguousarray(x2.astype(bf16))
    in_maps = []
    for e in range(E):
        # W1[e] [D, F] -> [P(p), NJ(j), KD(k), P(c)]
        w1s = np.ascontiguousarray(
            W1[e].reshape(KD, P, NJ, P).transpose(1, 2, 0, 3).astype(bf16))
        # W2[e] [F, D] -> [P(p), NJ(j), D]
        w2s = np.ascontiguousarray(
            W2[e].reshape(NJ, P, D).transpose(1, 0, 2).astype(bf16))
        in_maps.append({
            "x": x_b,
            "W1s": w1s,
            "W2s": w2s,
            "idx": np.ascontiguousarray(idx_e[e].reshape(NG, P).T),
            "wc": np.ascontiguousarray(w_e[e].reshape(NG, P).T),
            "b1t": np.ascontiguousarray(b1[e].reshape(NJ, P).T),
            "b2r": np.ascontiguousarray(np.broadcast_to(b2[e], (P, D))),
        })
    return in_maps


def kernel(x, W1, b1, W2, b2, Wg, bg, num_experts_per_token):
    from concourse.bass_utils import run_bass_kernel_spmd

    x2 = np.ascontiguousarray(np.asarray(x, np.float32).reshape(T, D))
    W1 = np.asarray(W1, np.float32)
    b1 = np.asarray(b1, np.float32)
    W2 = np.asarray(W2, np.float32)
    b2 = np.asarray(b2, np.float32)

    routing = _route(x2, Wg, bg)
    if routing is None or int(num_experts_per_token) != K_TOP:
        # capacity overflow or unexpected top-k: correct slow path
        gate = x2.astype(np.float64) @ np.asarray(Wg, np.float64) + np.asarray(bg, np.float64)
        k = int(num_experts_per_token)
        part = np.argsort(-gate, axis=1)[:, :k]
        sc = gate[np.arange(T)[:, None], part]
        sc = sc - sc.max(axis=1, keepdims=True)
        pr = np.exp(sc); pr /= pr.sum(axis=1, keepdims=True)
        out = np.zeros((T, D), np.float32)
        for e in range(E):
            mask = part == e
            tok = np.nonzero(mask.any(axis=1))[0]
            w = pr[mask].astype(np.float32)
            hcur = np.maximum(x2[tok] @ W1[e] + b1[e], 0.0)
            out[tok] += w[:, None] * (hcur @ W2[e] + b2[e])
        return out.reshape(B, S, D)

    idx_e, w_e, n_e = routing

    if "nc" not in _CACHE:
        _CACHE["nc"] = _build_program()
    nc = _CACHE["nc"]

    in_maps = _mk_in_maps(x2, W1, b1, W2, b2, idx_e, w_e)
    res = run_bass_kernel_spmd(nc, in_maps, list(range(E)))

    out = np.zeros((T, D), np.float32)
    for e in range(E):
        n = n_e[e]
        out[idx_e[e][:n]] += res.results[e]["yout"][:n]
    return out.reshape(B, S, D)


# revision 12
# speedup vs baseline: 1.0700x; 1.0212x over previous
"""Expert-parallel MoE feed-forward for Trainium2 (8 NeuronCores).

Strategy (v4):
  - Host: gate + top-2 routing (0.02% of FLOPs), builds per-expert token
    index lists.  Expert e is owned by core e.
  - Device (same SPMD program on all 8 cores): indirect-DMA gather of the
    expert's tokens (bf16), DMA-XBAR transpose to [d, tok] layout, FFN
    y = relu(x@W1+b1)@W2+b2 in bf16 (full PE rate, fp32 PSUM accumulate),
    scale by combine weight, write compact [C, D] fp32 result.
  - Host: scatter-add compact results into the [B,S,D] output.

Design notes:
  - bf16 operands everywhere on the PE (same 1 cycle/row as f32r, half the
    DMA/SBUF footprint) -> single pass over all C tokens, so W1 and W2
    stream from HBM exactly once (16MB instead of 64MB).
  - x transposes on the DMA XBAR (dma_start_transpose); PE runs only matmuls.
  - W2 resident in SBUF (64KB/partition), loaded behind the gathers on the
    pool queue -> mm2 has no DMA dependence.
  - PE stationary operand changes on every matmul (consecutive reuse of the
    same stationary measures ~2x per-instruction overhead on TRN2).
  - mm1 PSUM drains on the vector engine: one fused (acc+b1) max 0 per chunk.
  - C = 1088 (max expert load this routing is 1075): third mm1 chunk is 64
    wide, last mm2 token tile is 64 tall.
  - Disjoint PSUM tags: mm1 accumulators (4 banks) + mm2 accumulators (4).

Layouts (per expert e / core e):
  xT[p, k, t]    = x[tok(t), k*128+p]          (DMA-XBAR-transposed gather)
  h [p, j, t]    = relu(x @ W1 + b1)[tok(t), j*128+p]
  w1_d[p, j, k, c] = W1[e][k*128+p, j*128+c]   (host-swizzled, bf16)
  w2_d[p, j, d]  = W2[e][j*128+p, d]           (host-swizzled, bf16)
  mm1: h[:, j, chunk] = sum_k w1t[:, k, :].T @ xT[:, k, chunk]
  mm2: y[m-tile, dhalf] = sum_j h[:, j, m-tile].T @ w2sb[:, j, dhalf]
"""

import numpy as np

B, S, D, F, E = 2, 2048, 1024, 4096, 8
T = B * S                      # 4096 tokens
K_TOP = 2
C = 1088                       # per-expert token capacity (8.5 * 128)
P = 128
KD = D // P                    # 8  k-tiles (mm1 contraction)
NJ = F // P                    # 32 f-tiles
NG = 9                         # gather tiles of 128 (covers C with padding)
DH = D // 2                    # 512 (mm2 moving width)
CHUNKS = [(0, 512), (512, 512), (1024, 64)]    # mm1 token chunks
MTILES = [(m * P, P) for m in range(8)] + [(1024, 64)]  # mm2 token tiles

_CACHE = {}


def _build_program(reps=1, loop_n=1):
    import concourse.bass as bass
    import concourse.mybir as mybir
    import concourse.tile as tile
    from concourse import bacc
    from contextlib import ExitStack

    f32 = mybir.dt.float32
    bf16 = mybir.dt.bfloat16
    i32 = mybir.dt.int32

    nc = bacc.Bacc("TRN2", target_bir_lowering=False, debug=False)

    x_d = nc.dram_tensor("x", [T, D], bf16, kind="ExternalInput").ap()
    w1_d = nc.dram_tensor("W1s", [P, NJ, KD, P], bf16, kind="ExternalInput").ap()
    w2_d = nc.dram_tensor("W2s", [P, NJ, D], bf16, kind="ExternalInput").ap()
    idx_d = nc.dram_tensor("idx", [P, NG], i32, kind="ExternalInput").ap()
    wc_d = nc.dram_tensor("wc", [P, NG], f32, kind="ExternalInput").ap()
    b1_d = nc.dram_tensor("b1t", [P, NJ], f32, kind="ExternalInput").ap()
    # b2 replicated across partitions for the free-axis bias add
    b2_d = nc.dram_tensor("b2r", [P, D], f32, kind="ExternalInput").ap()
    y_d = nc.dram_tensor("yout", [C, D], f32, kind="ExternalOutput").ap()

    with tile.TileContext(nc) as tc, ExitStack() as ctx:
        sb = ctx.enter_context(tc.tile_pool(name="sb", bufs=1))
        ps = ctx.enter_context(tc.tile_pool(name="ps", bufs=1, space="PSUM"))

        idx_t = sb.tile([P, NG], i32, tag="idx")
        wc_t = sb.tile([P, NG], f32, tag="wc")
        b1_t = sb.tile([P, NJ], f32, tag="b1")
        b2_t = sb.tile([P, D], f32, tag="b2")
        nc.sync.dma_start(idx_t[:], idx_d[:])
        nc.sync.dma_start(wc_t[:], wc_d[:])
        nc.sync.dma_start(b1_t[:], b1_d[:])
        nc.sync.dma_start(b2_t[:], b2_d[:])

        # W2 resident in SBUF for the whole kernel (64KB/partition)
        w2sb = sb.tile([P, NJ, D], bf16, tag="w2r")

        loop_cm = tc.For_i(0, loop_n, 1) if loop_n > 1 else None
        if loop_cm is not None:
            loop_cm.__enter__()

        for rep in range(reps):
            # --- gather + DMA-XBAR transpose ---
            xT = sb.tile([P, KD, NG * P], bf16, tag="xT", bufs=1,
                         name=f"xT_{rep}")
            for g in range(NG):
                xg = sb.tile([P, D], bf16, tag="xg", bufs=5,
                             name=f"xg_{rep}_{g}")
                nc.gpsimd.indirect_dma_start(
                    out=xg[:], out_offset=None,
                    in_=x_d[:],
                    in_offset=bass.IndirectOffsetOnAxis(
                        ap=idx_t[:, g:g + 1], axis=0),
                )
                for k in range(KD):
                    nc.scalar.dma_start_transpose(
                        xT[:, k, g * P:(g + 1) * P],
                        xg[:, k * P:(k + 1) * P])

            if rep == 0:
                # behind the gathers on the pool queue; needed only by mm2
                for q in range(4):
                    nc.gpsimd.dma_start(w2sb[:, q * 8:(q + 1) * 8, :],
                                        w2_d[:, q * 8:(q + 1) * 8, :])

            # --- mm1 + relu:  h2[:, m, j, t] = relu(sum_k w1.T @ xT + b1) ---
            # h is stored m-tile-major so each mm2 token tile's 32 stationary
            # blocks occupy one contiguous 8KB/partition window (large
            # stationary footprints measure ~4% slower PE streaming).
            h = sb.tile([P, NG, NJ, P], bf16, tag="h", bufs=1, name=f"h_{rep}")
            for j in range(NJ):
                w1t = sb.tile([P, KD, P], bf16, tag="w1", bufs=12,
                              name=f"w1_{rep}_{j}")
                nc.sync.dma_start(w1t[:], w1_d[:, j])
                accs = []
                for ci, (t0, tn) in enumerate(CHUNKS):
                    accs.append(ps.tile([P, tn], f32, tag="p1", bufs=4,
                                        name=f"p1_{rep}_{j}_{ci}",
                                        padded_shape=[P, 512]))
                # k inner so the PE stationary changes on every matmul
                for ci, (t0, tn) in enumerate(CHUNKS):
                    for k in range(KD):
                        nc.tensor.matmul(
                            accs[ci][:],
                            lhsT=w1t[:, k, :],
                            rhs=xT[:, k, t0:t0 + tn],
                            start=(k == 0), stop=(k == KD - 1))
                for ci, (t0, tn) in enumerate(CHUNKS):
                    # fused (acc + b1) max 0 -> bf16, one drain per m-tile
                    for mr in range(tn // P if tn >= P else 1):
                        mi = t0 // P + mr
                        mw = min(P, tn)
                        nc.vector.tensor_scalar(
                            out=h[:, mi, j, 0:mw],
                            in0=accs[ci][:, mr * P:mr * P + mw],
                            scalar1=b1_t[:, j:j + 1], scalar2=0.0,
                            op0=mybir.AluOpType.add, op1=mybir.AluOpType.max)

            # --- mm2: y[m-tile, :] = sum_j h2[:, m, j, :].T @ w2sb[:, j, :] ---
            for mi, (m0, mn) in enumerate(MTILES):
                acc2 = [ps.tile([mn, DH], f32, tag="p2", bufs=4,
                                name=f"p2_{rep}_{mi}_{dn}",
                                padded_shape=[P, DH]) for dn in range(2)]
                # j inner: stationary (h block) changes on every matmul
                for dn in range(2):
                    for j in range(NJ):
                        nc.tensor.matmul(
                            acc2[dn][:],
                            lhsT=h[:, mi, j, 0:mn],
                            rhs=w2sb[:, j, dn * DH:(dn + 1) * DH],
                            start=(j == 0), stop=(j == NJ - 1))
                for dn in range(2):
                    ot = sb.tile([mn, DH], f32, tag="ot", bufs=4,
                                 name=f"ot_{rep}_{mi}_{dn}",
                                 padded_shape=[P, DH])
                    nc.vector.tensor_tensor(
                        out=ot[:], in0=acc2[dn][:],
                        in1=b2_t[:mn, dn * DH:(dn + 1) * DH],
                        op=mybir.AluOpType.add)
                    nc.vector.tensor_scalar_mul(
                        ot[:], ot[:], wc_t[:mn, mi:mi + 1])
                    nc.scalar.dma_start(
                        y_d[m0:m0 + mn, dn * DH:(dn + 1) * DH], ot[:])

        if loop_cm is not None:
            loop_cm.__exit__(None, None, None)

    nc.compile()
    return nc


def _route(x2, Wg, bg):
    """Host-side top-2 routing in float64 (stable ordering)."""
    gate = x2.astype(np.float64) @ np.asarray(Wg, np.float64) + np.asarray(bg, np.float64)
    part = np.argpartition(-gate, K_TOP - 1, axis=1)[:, :K_TOP]      # [T, 2]
    rows = np.arange(T)[:, None]
    sc = gate[rows, part]                                            # [T, 2]
    sc = sc - sc.max(axis=1, keepdims=True)
    e_sc = np.exp(sc)
    probs = e_sc / e_sc.sum(axis=1, keepdims=True)                   # [T, 2]
    idx_e, w_e, n_e = [], [], []
    for e in range(E):
        mask = part == e                                             # [T, 2]
        tok = np.nonzero(mask.any(axis=1))[0]
        pr = probs[mask]                                             # aligned with tok
        n = len(tok)
        pad = NG * P - n
        if n > C:
            return None                                              # capacity overflow
        idx_e.append(np.concatenate([tok, np.zeros(pad, np.int64)]).astype(np.int32))
        w_e.append(np.concatenate([pr, np.zeros(pad)]).astype(np.float32))
        n_e.append(n)
    return idx_e, w_e, n_e


def _mk_in_maps(x2, W1, b1, W2, b2, idx_e, w_e):
    """Build per-core device input dicts (bf16 swizzled weights)."""
    import ml_dtypes
    bf16 = ml_dtypes.bfloat16

    x_b = np.ascontiguousarray(x2.astype(bf16))
    in_maps = []
    for e in range(E):
        # W1[e] [D, F] -> [P(p), NJ(j), KD(k), P(c)]
        w1s = np.ascontiguousarray(
            W1[e].reshape(KD, P, NJ, P).transpose(1, 2, 0, 3).astype(bf16))
        # W2[e] [F, D] -> [P(p), NJ(j), D]
        w2s = np.ascontiguousarray(
            W2[e].reshape(NJ, P, D).transpose(1, 0, 2).astype(bf16))
        in_maps.append({
            "x": x_b,
            "W1s": w1s,
            "W2s": w2s,
            "idx": np.ascontiguousarray(idx_e[e].reshape(NG, P).T),
            "wc": np.ascontiguousarray(w_e[e].reshape(NG, P).T),
            "b1t": np.ascontiguousarray(b1[e].reshape(NJ, P).T),
            "b2r": np.ascontiguousarray(np.broadcast_to(b2[e], (P, D))),
        })
    return in_maps


def kernel(x, W1, b1, W2, b2, Wg, bg, num_experts_per_token):
    from concourse.bass_utils import run_bass_kernel_spmd

    x2 = np.ascontiguousarray(np.asarray(x, np.float32).reshape(T, D))
    W1 = np.asarray(W1, np.float32)
    b1 = np.asarray(b1, np.float32)
    W2 = np.asarray(W2, np.float32)
    b2 = np.asarray(b2, np.float32)

    routing = _route(x2, Wg, bg)
    if routing is None or int(num_experts_per_token) != K_TOP:
        # capacity overflow or unexpected top-k: correct slow path
        gate = x2.astype(np.float64) @ np.asarray(Wg, np.float64) + np.asarray(bg, np.float64)
        k = int(num_experts_per_token)
        part = np.argsort(-gate, axis=1)[:, :k]
        sc = gate[np.arange(T)[:, None], part]
        sc = sc - sc.max(axis=1, keepdims=True)
        pr = np.exp(sc); pr /= pr.sum(axis=1, keepdims=True)
        out = np.zeros((T, D), np.float32)
        for e in range(E):
            mask = part == e
            tok = np.nonzero(mask.any(axis=1))[0]
            w = pr[mask].astype(np.float32)
            hcur = np.maximum(x2[tok] @ W1[e] + b1[e], 0.0)
            out[tok] += w[:, None] * (hcur @ W2[e] + b2[e])
        return out.reshape(B, S, D)

    idx_e, w_e, n_e = routing

    if "nc" not in _CACHE:
        _CACHE["nc"] = _build_program()
    nc = _CACHE["nc"]

    in_maps = _mk_in_maps(x2, W1, b1, W2, b2, idx_e, w_e)
    res = run_bass_kernel_spmd(nc, in_maps, list(range(E)))

    out = np.zeros((T, D), np.float32)
    for e in range(E):
        n = n_e[e]
        out[idx_e[e][:n]] += res.results[e]["yout"][:n]
    return out.reshape(B, S, D)
